# revision 47
# baseline (speedup 1.0000x reference)
"""GAT (2-layer, global-softmax attention) Trainium2 Bass kernel, 8-core SPMD.

Sharding: core c in [0..3] handles batch 0, source-node block j0 = 128*c;
cores [4..7] handle batch 1. Each core computes eT[j_shard, i] for its
128-row block of source nodes against all N=512 destination nodes, the
masked exp, and the partial aggregation U_c = h_shard^T-weighted sums.
A 4-core AllReduce per batch group combines U_c and the softmax
denominator partials (the reference softmaxes over ALL N^2 edges, so the
denominator is a single scalar per batch).

Math trick for the edge scores: with z = relu(s_i[i,k] + s_j[j,k] + b[k]),
e[i,j] = sum_k z[i,j,k]*a2[k]. Fold |a2[k]| into the attention weights
(a2*relu(x) = sign(a2)*relu(|a2|*x)) and sort k so positive signs come
first. Per k, a rank-2 TensorE matmul ([s_j_col; 1]^T @ [1; s_i_row])
produces the (128,512) score slab in PSUM (pairs share a 2-bank tile),
ScalarE relu's each pair contiguously into bf16 slab tiles, and VectorE
contracts over k with in-tile pairwise adds (bf16 2x mode), summing the
positive and negative sign groups separately and subtracting.
"""

import sys

if "/opt/trn_rl_repo" not in sys.path:
    sys.path.insert(0, "/opt/trn_rl_repo")

import numpy as np
import ml_dtypes

import concourse.bass as bass
import concourse.mybir as mybir
import concourse.tile as tile
from concourse import bacc
from concourse.bass_utils import run_bass_kernel_spmd

BF16 = mybir.dt.bfloat16
F32 = mybir.dt.float32
AF = mybir.ActivationFunctionType
ALU = mybir.AluOpType

B, N, IN_DIM, MEM, HID = 2, 512, 512, 300, 64
P = 128  # j-shard rows per core
NCORES = 8
GROUPS = [[0, 1, 2, 3], [4, 5, 6, 7]]
NEG_SLOPE = 0.01
MASK_OFF = 30.0  # masked logits get exp(x*0 - 30) ~ 9e-14 instead of exp(-1e30)=0

KT0 = [128, 128, 128, 128]  # layer-0 contraction tiles over IN_DIM=512
KT1 = [128, 128, 44]  # layer-1 contraction tiles over MEM=300
MC = [128, 128, 44]  # chunks of MEM=300 (output feature dim)
MJ = [128, 128, 45]  # chunks of MEM+1=301 (a1J with bias row appended)
NJC = N // P  # 4 j-chunks


def _gat_layer(nc, tc, pools, lay, fT, ktsz, Wt, bct, brt, cst, p_pos):
    """Emit one GAT layer. fT: [128, nkt, 512] tiles of f^T (feature-major).

    Returns Uall ([128,3,512] f32 tiles of the un-normalized aggregation)
    and rS ([128,1] f32, reciprocal of the global softmax denominator).
    """
    const, work, mp, zp, tp, dram = pools
    nkt = len(ktsz)
    a1It, a1Jt, jselt, adjt, identt, onest = (
        cst["a1It"], cst["a1Jt"], cst["jselt"], cst["adjt"], cst["identt"],
        cst["onest"],
    )

    # ---- hT[m', i] = sum_m W[m, m'] fT[m, i]  (+ bias per-partition) ----
    hT = work.tile([128, 3, 512], BF16, tag="hT")
    for mc in range(3):
        msz, mo = MC[mc], mc * 128
        ps = mp.tile([128, 512], F32, tag="mm")
        for kt in range(nkt):
            ks = ktsz[kt]
            nc.tensor.matmul(
                ps[:msz, :],
                Wt[:ks, kt, mo : mo + msz],
                fT[:ks, kt, :],
                start=(kt == 0),
                stop=(kt == nkt - 1),
            )
        nc.scalar.activation(
            hT[:msz, mc, :], ps[:msz, :], AF.Identity, bias=bct[:msz, mc : mc + 1]
        )

    # ---- h[j, m'] = sum_m fT[m, j] W[m, m'] + b  (bias via K=1 ones matmul) ----
    h = work.tile([128, NJC, 300], BF16, tag="h")
    for jc in range(NJC):
        ps = mp.tile([128, 512], F32, tag="mm")
        for kt in range(nkt):
            ks = ktsz[kt]
            nc.tensor.matmul(
                ps[:, :300],
                fT[:ks, kt, jc * 128 : (jc + 1) * 128],
                Wt[:ks, kt, :],
                start=(kt == 0),
                stop=False,
            )
        nc.tensor.matmul(
            ps[:, :300], onest[0:1, :128], brt[0:1, :], start=False, stop=True
        )
        nc.vector.tensor_copy(h[:, jc, :], ps[:, :300])

    # ---- siT[k, i] = sum_m a1I~[m, k] hT[m, i]  (|a2| pre-folded) ----
    siT = work.tile([64, 512], BF16, tag="siT")
    ps = mp.tile([128, 512], F32, tag="mm")
    for kt in range(3):
        ks = MC[kt]
        nc.tensor.matmul(
            ps[:64, :], a1It[:ks, kt, :], hT[:ks, kt, :],
            start=(kt == 0), stop=(kt == 2),
        )
    nc.vector.tensor_copy(siT[:, :], ps[:64, :])

    # ---- h_shard[j', m] = sum_j jselT[j, j'] h[j, m]  (one-hot row select) ----
    hs = work.tile([128, 300], BF16, tag="hs")
    ps = mp.tile([128, 512], F32, tag="mm")
    for kt in range(NJC):
        nc.tensor.matmul(
            ps[:, :300], jselt[:, kt, :], h[:, kt, :],
            start=(kt == 0), stop=(kt == NJC - 1),
        )
    nc.vector.tensor_copy(hs[:, :], ps[:, :300])

    # ---- h_shardT ----
    hsT = work.tile([128, 3, 128], BF16, tag="hsT")
    for mc in range(3):
        msz, mo = MC[mc], mc * 128
        pt = tp.tile([128, 128], BF16, tag="tp")
        nc.tensor.transpose(pt[:msz, :], hs[:, mo : mo + msz], identt[:, :])
        nc.vector.tensor_copy(hsT[:msz, mc, :], pt[:msz, :])

    # ---- sjT[k, j'] = sum_m a1J~[m, k] hsT[m, j'] + a1b~ (K=1 ones matmul) ----
    sjT = work.tile([64, 128], BF16, tag="sjT")
    ps = mp.tile([128, 512], F32, tag="mm")
    for kt in range(3):
        ks = MC[kt]
        nc.tensor.matmul(
            ps[:64, :128], a1Jt[:ks, kt, :], hsT[:ks, kt, :],
            start=(kt == 0), stop=False,
        )
    nc.tensor.matmul(
        ps[:64, :128], cst["a1brt"][0:1, :], onest[0:1, :128],
        start=False, stop=True,
    )
    nc.vector.tensor_copy(sjT[:, :], ps[:64, :128])

    # ---- flatten to k-major rows + ones rows for the rank-2 produce MMs ----
    lhsJ = work.tile([2, 64 * 128], BF16, tag="lhsJ")
    rhsA = work.tile([2, 64 * 512], BF16, tag="rhsA")
    nc.gpsimd.dma_start(out=lhsJ[1:2, :], in_=cst["d_ones"][0:1, 0 : 64 * 128])
    nc.scalar.dma_start(out=lhsJ[0:1, :], in_=sjT[:, :])
    nc.gpsimd.dma_start(out=rhsA[0:1, :], in_=cst["d_ones"][0:1, :])
    nc.sync.dma_start(out=rhsA[1:2, :], in_=siT[:, :])

    # ---- main loop: rank-2 produce MMs (pairs into a 2-bank PSUM tile) ->
    # one ScalarE relu per pair (contiguous writes, FD=1024). k-contraction
    # via bf16 pairwise in-tile adds (VectorE 2x mode, contiguous); R is
    # split into 4 tiles so tree adds overlap the remaining relu stream.
    # Sign handling: pos k's in [0, p_pos), neg in [p_pos, 64); per-tile
    # sign-pure partial sums, combined as sum(pos) - sum(neg) at the end.
    RT, RK = 8, HID // 8  # 8 tiles x 8 slabs
    Rs = [
        work.tile([128, RK, 512], BF16, tag=f"R{t}", name=f"R{t}_{lay}")
        for t in range(RT)
    ]
    for kp in range(HID // 2):
        z = zp.tile([128, 2, 512], F32, tag="z")
        for h in range(2):
            k = 2 * kp + h
            nc.tensor.matmul(
                z[:, h, :],
                lhsJ[:, k * 128 : (k + 1) * 128],
                rhsA[:, k * 512 : (k + 1) * 512],
                start=True,
                stop=True,
            )
        k0 = 2 * kp
        r_out = Rs[k0 // RK][:, k0 % RK : k0 % RK + 2, :]
        if kp % 8 == 5:
            # offload ~1/8 of the relu stream to VectorE (ScalarE is the
            # main-loop rate limiter; DVE has slack before its tree adds)
            nc.vector.tensor_scalar(r_out, z[:, :, :], 0.0, None, op0=ALU.max)
        else:
            nc.scalar.activation(r_out, z[:, :, :], AF.Relu)

    def tree_sum(tile_, lo, hi):
        """In-tile pairwise bf16 tree over slab range [lo, hi); returns slab
        AP holding the sum (accumulated into slab lo)."""
        idxs = list(range(lo, hi))
        while len(idxs) > 1:
            nxt = []
            for a in range(0, len(idxs) - 1, 2):
                i0, i1 = idxs[a], idxs[a + 1]
                nc.vector.tensor_add(
                    tile_[:, i0, :], tile_[:, i0, :], tile_[:, i1, :]
                )
                nxt.append(i0)
            if len(idxs) % 2:
                nxt.append(idxs[-1])
            idxs = nxt
        return tile_[:, idxs[0], :]

    pos_parts, neg_parts = [], []
    for t in range(RT):
        lo_k, hi_k = t * RK, (t + 1) * RK
        if p_pos >= hi_k:
            pos_parts.append(tree_sum(Rs[t], 0, RK))
        elif p_pos <= lo_k:
            neg_parts.append(tree_sum(Rs[t], 0, RK))
        else:
            sp = p_pos - lo_k
            pos_parts.append(tree_sum(Rs[t], 0, sp))
            neg_parts.append(tree_sum(Rs[t], sp, RK))

    def combine(parts, tag):
        acc = work.tile([128, 512], F32, tag=tag)
        if not parts:
            nc.vector.memset(acc[:, :], 0.0)
        elif len(parts) == 1:
            nc.vector.tensor_copy(acc[:, :], parts[0])
        else:
            nc.vector.tensor_add(acc[:, :], parts[0], parts[1])
            for p_ in parts[2:]:
                nc.vector.tensor_add(acc[:, :], acc[:, :], p_)
        return acc

    e_pos = combine(pos_parts, "epos")
    e_neg = combine(neg_parts, "eneg")

    # ---- epilogue: +a2_b, leaky-relu, mask, exp (+ row-sum partials) ----
    e_c = work.tile([128, 512], F32, tag="ec")
    nc.vector.tensor_sub(e_c[:, :], e_pos[:, :], e_neg[:, :])
    e_s = work.tile([128, 512], F32, tag="es")
    nc.scalar.activation(e_s[:, :], e_c[:, :], AF.Identity, bias=cst["a2bt"][:, :])
    lr = work.tile([128, 512], F32, tag="lr")
    nc.vector.scalar_tensor_tensor(
        lr[:, :], e_s[:, :], NEG_SLOPE, e_s[:, :], op0=ALU.mult, op1=ALU.max
    )
    tm = work.tile([128, 512], F32, tag="tm")
    nc.vector.scalar_tensor_tensor(
        tm[:, :], lr[:, :], MASK_OFF, adjt[:, :], op0=ALU.add, op1=ALU.mult
    )
    E = work.tile([128, 512], BF16, tag="E")
    sE = work.tile([128, 1], F32, tag="sE")
    nc.scalar.activation(
        E[:, :], tm[:, :], AF.Exp, bias=cst["moff"][:, :], accum_out=sE[:, :]
    )

    # ---- partial aggregation U_c[m, i] = sum_j' hs[j', m] E[j', i] ----
    # bf16 collective payload: rows 0:300 carry U, row 300 cols 0:128
    # carry the per-partition denominator partials (cols 128: zeroed).
    ccU_in = dram.tile([301, 512], BF16, tag=f"ccU_in{lay}")
    ccU_out = dram.tile([301, 512], BF16, tag=f"ccU_out{lay}")
    dma_engs = [nc.sync, nc.scalar, nc.gpsimd, nc.sync]
    for mc in range(3):
        msz, mo = MC[mc], mc * 128
        pu = mp.tile([128, 512], F32, tag="mm")
        nc.tensor.matmul(
            pu[:msz, :], hs[:, mo : mo + msz], E[:, :], start=True, stop=True
        )
        ust = work.tile([128, 512], BF16, tag=f"ust{mc}", name=f"ust{mc}_{lay}")
        nc.vector.tensor_copy(ust[:msz, :], pu[:msz, :])
        # split each chunk across two DMA queues (different engines)
        h0 = (msz + 1) // 2
        if h0 % 32:
            h0 = 64 if msz > 64 else msz
        dma_engs[(2 * mc) % 4].dma_start(
            out=ccU_in[mo : mo + h0, :], in_=ust[:h0, :]
        )
        if h0 < msz:
            dma_engs[(2 * mc + 1) % 4].dma_start(
                out=ccU_in[mo + h0 : mo + msz, :], in_=ust[h0:msz, :]
            )
    # sE (128,1) -> PE transpose -> single-descriptor (1,128) row write
    sEb = work.tile([128, 1], BF16, tag="sEb")
    nc.vector.tensor_copy(sEb[:, :], sE[:, :])
    pt = tp.tile([128, 128], BF16, tag="tp")
    nc.tensor.transpose(pt[:1, :128], sEb[:, :], identt[:, :])
    sEr = work.tile([1, 128], BF16, tag="sEr")
    nc.vector.tensor_copy(sEr[:, :], pt[:1, :128])
    zrow = work.tile([1, 512], BF16, tag="zrow")
    nc.vector.memset(zrow[:, :], 0.0)
    nc.sync.dma_start(out=ccU_in[300:301, :], in_=zrow[:, :])
    nc.sync.dma_start(out=ccU_in[300:301, 0:128], in_=sEr[:, :])

    nc.gpsimd.collective_compute(
        "AllReduce",
        ALU.add,
        replica_groups=GROUPS,
        ins=[ccU_in.opt()],
        outs=[ccU_out.opt()],
    )

    # ---- back: global denominator S, broadcast 1/S to all partitions ----
    Uall = work.tile([128, 3, 512], BF16, tag="Uall")
    for mc in range(3):
        msz, mo = MC[mc], mc * 128
        h0 = 64 if msz > 64 else msz
        dma_engs[(2 * mc) % 4].dma_start(
            out=Uall[:h0, mc, :], in_=ccU_out[mo : mo + h0, :]
        )
        if h0 < msz:
            dma_engs[(2 * mc + 1) % 4].dma_start(
                out=Uall[h0:msz, mc, :], in_=ccU_out[mo + h0 : mo + msz, :]
            )
    sEgr = work.tile([1, 128], BF16, tag="sEgr")
    nc.sync.dma_start(out=sEgr[:, :], in_=ccU_out[300:301, 0:128])
    ptb = tp.tile([128, 128], BF16, tag="tp")
    nc.tensor.transpose(ptb[:128, 0:1], sEgr[:, :], identt[0:1, 0:1])
    sEg = work.tile([128, 1], BF16, tag="sEg")
    nc.vector.tensor_copy(sEg[:, :], ptb[:128, 0:1])
    pS = mp.tile([128, 512], F32, tag="mm")
    nc.tensor.matmul(pS[:, :1], onest[:, :], sEg[:, :], start=True, stop=True)
    rS = work.tile([128, 1], F32, tag="rS")
    nc.vector.reciprocal(rS[:, :], pS[:, :1])
    return Uall, rS


def _build(p_pos, a2b, debug):
    nc = bacc.Bacc(
        "TRN2",
        target_bir_lowering=False,
        debug=debug,
        num_devices=NCORES,
    )
    # Inputs are host-pre-tiled to (128, nkt*width) so each const load is a
    # single 2D DMA with 128 fat contiguous descriptors.
    d_fT0 = nc.dram_tensor("fT0", [128, 4 * N], BF16, kind="ExternalInput")
    d_adjT = nc.dram_tensor("adjTm", [P, N], F32, kind="ExternalInput")
    d_jselT = nc.dram_tensor("jselT", [128, 4 * P], BF16, kind="ExternalInput")
    d_w0 = nc.dram_tensor("w0b", [128, 4 * 300], BF16, kind="ExternalInput")
    d_w1 = nc.dram_tensor("w1b", [128, 3 * 300], BF16, kind="ExternalInput")
    d_a1I = nc.dram_tensor("a1Ib", [128, 3 * 64], BF16, kind="ExternalInput")
    d_a1J = nc.dram_tensor("a1Jpb", [128, 3 * 64], BF16, kind="ExternalInput")
    d_a1br = nc.dram_tensor("a1br", [1, 64], BF16, kind="ExternalInput")
    d_b0c = nc.dram_tensor("b0c", [128, 3], F32, kind="ExternalInput")
    d_b1c = nc.dram_tensor("b1c", [128, 3], F32, kind="ExternalInput")
    d_b0r = nc.dram_tensor("b0r", [1, 300], BF16, kind="ExternalInput")
    d_b1r = nc.dram_tensor("b1r", [1, 300], BF16, kind="ExternalInput")
    d_id = nc.dram_tensor("ident", [128, 128], BF16, kind="ExternalInput")
    d_ones = nc.dram_tensor("onesb", [1, 64 * 512], BF16, kind="ExternalInput")
    d_out = nc.dram_tensor("outT", [300, N], F32, kind="ExternalOutput")

    with tile.TileContext(nc) as tc:
        with (
            tc.tile_pool(name="const", bufs=1) as const,
            tc.tile_pool(name="work", bufs=1) as work,
            tc.tile_pool(name="mp", bufs=3, space="PSUM") as mp,
            tc.tile_pool(name="zp", bufs=2, space="PSUM") as zp,
            tc.tile_pool(name="tp", bufs=1, space="PSUM") as tp,
            tc.tile_pool(name="dram", bufs=1, space="DRAM") as dram,
        ):
            fT = const.tile([128, 4, 512], BF16, tag="fT")
            nc.sync.dma_start(fT[:, :, :], d_fT0[:, :])
            w0t = const.tile([128, 4, 300], BF16, tag="w0t")
            nc.sync.dma_start(w0t[:, :, :], d_w0[:, :])
            w1t = const.tile([128, 3, 300], BF16, tag="w1t")
            nc.sync.dma_start(w1t[:, :, :], d_w1[:, :])
            a1It = const.tile([128, 3, 64], BF16, tag="a1It")
            nc.sync.dma_start(a1It[:, :, :], d_a1I[:, :])
            a1Jt = const.tile([128, 3, 64], BF16, tag="a1Jt")
            nc.sync.dma_start(a1Jt[:, :, :], d_a1J[:, :])
            a1brt = const.tile([1, 64], BF16, tag="a1brt")
            nc.sync.dma_start(a1brt[:, :], d_a1br[:, :])
            jselt = const.tile([128, 4, 128], BF16, tag="jselt")
            nc.sync.dma_start(jselt[:, :, :], d_jselT[:, :])
            adjt = const.tile([128, 512], F32, tag="adjt")
            nc.sync.dma_start(adjt[:, :], d_adjT[:, :])
            b0ct = const.tile([128, 3], F32, tag="b0ct")
            nc.sync.dma_start(b0ct[:, :], d_b0c[:, :])
            b1ct = const.tile([128, 3], F32, tag="b1ct")
            nc.sync.dma_start(b1ct[:, :], d_b1c[:, :])
            b0rt = const.tile([1, 300], BF16, tag="b0rt")
            nc.sync.dma_start(b0rt[:, :], d_b0r[:, :])
            b1rt = const.tile([1, 300], BF16, tag="b1rt")
            nc.sync.dma_start(b1rt[:, :], d_b1r[:, :])
            identt = const.tile([128, 128], BF16, tag="identt")
            nc.sync.dma_start(identt[:, :], d_id[:, :])
            onest = const.tile([128, 128], BF16, tag="onest")
            nc.vector.memset(onest[:, :], 1.0)
            a2bt = const.tile([128, 1], F32, tag="a2bt")
            nc.vector.memset(a2bt[:, :], a2b)
            moff = const.tile([128, 1], F32, tag="moff")
            nc.vector.memset(moff[:, :], -MASK_OFF)

            cst = dict(
                a1It=a1It, a1Jt=a1Jt, a1brt=a1brt, jselt=jselt, adjt=adjt,
                identt=identt, onest=onest, a2bt=a2bt, moff=moff, d_ones=d_ones,
            )
            pools = (const, work, mp, zp, tp, dram)

            U1, rS1 = _gat_layer(
                nc, tc, pools, 0, fT, KT0, w0t, b0ct, b0rt, cst, p_pos
            )
            f1T = work.tile([128, 3, 512], BF16, tag="f1T")
            for mc in range(3):
                msz = MC[mc]
                nc.scalar.activation(
                    f1T[:msz, mc, :], U1[:msz, mc, :], AF.Copy,
                    bias=0.0, scale=rS1[:msz, :],
                )

            U2, rS2 = _gat_layer(
                nc, tc, pools, 1, f1T, KT1, w1t, b1ct, b1rt, cst, p_pos
            )
            out_engs = [nc.sync, nc.scalar, nc.gpsimd]
            for mc in range(3):
                msz, mo = MC[mc], mc * 128
                st = work.tile(
                    [128, 512], F32, tag=f"fout{mc}", name=f"fout{mc}"
                )
                nc.scalar.activation(
                    st[:msz, :], U2[:msz, mc, :], AF.Copy,
                    bias=0.0, scale=rS2[:msz, :],
                )
                h0 = 64 if msz > 64 else msz
                out_engs[mc].dma_start(
                    out=d_out[mo : mo + h0, :], in_=st[:h0, :]
                )
                if h0 < msz:
                    out_engs[(mc + 1) % 3].dma_start(
                        out=d_out[mo + h0 : mo + msz, :], in_=st[h0:msz, :]
                    )

    nc.compile()
    return nc


_CACHE = {}


def _get_program(p_pos, a2b, debug=False):
    key = (p_pos, float(a2b), debug)
    if key not in _CACHE:
        _CACHE[key] = _build(p_pos, float(a2b), debug)
    return _CACHE[key]


def _prep_inputs(feature, adj, w0, b0, w1, b1, a1_w, a1_b, a2_w, a2_b):
    """Host-side packing: dtype casts, |a2| fold, sign sort, shard slices."""
    bf = ml_dtypes.bfloat16
    a2 = np.asarray(a2_w, np.float32).reshape(-1)  # (64,)
    order = np.argsort((a2 < 0).astype(np.int32), kind="stable")
    p_pos = int((a2 >= 0).sum())
    absa2 = np.abs(a2[order])  # (64,)
    a1s = np.asarray(a1_w, np.float32)[:, order] * absa2[None, :]  # (600, 64)
    a1bs = (np.asarray(a1_b, np.float32)[order] * absa2)[None, :]  # (1, 64)
    def pack_tiles(arr, nkt):
        """(rows, w) -> (128, nkt*w): row t*128+p lands at [p, t*w : (t+1)*w],
        zero-padding rows to nkt*128."""
        rows, w = arr.shape
        padded = np.zeros((nkt * 128, w), np.float32)
        padded[:rows] = arr
        return np.ascontiguousarray(
            padded.reshape(nkt, 128, w).transpose(1, 0, 2).reshape(128, nkt * w)
        )

    a1I = pack_tiles(a1s[:MEM], 3).astype(bf)  # (128, 192)
    a1Jp = pack_tiles(a1s[MEM:], 3).astype(bf)  # (128, 192)
    a1br = a1bs.astype(bf)  # (1, 64)

    w0b = pack_tiles(np.asarray(w0, np.float32), 4).astype(bf)  # (128, 1200)
    w1b = pack_tiles(np.asarray(w1, np.float32), 3).astype(bf)  # (128, 900)
    b0c = np.zeros((128, 3), np.float32)
    b1c = np.zeros((128, 3), np.float32)
    b0f = np.asarray(b0, np.float32)
    b1f = np.asarray(b1, np.float32)
    for mc in range(3):
        b0c[: MC[mc], mc] = b0f[mc * 128 : mc * 128 + MC[mc]]
        b1c[: MC[mc], mc] = b1f[mc * 128 : mc * 128 + MC[mc]]
    b0r = b0f[None, :].astype(bf)
    b1r = b1f[None, :].astype(bf)
    ident = np.eye(128, dtype=np.float32).astype(bf)

    featT = [
        pack_tiles(np.asarray(feature[b], np.float32).T, 4).astype(bf)
        for b in range(B)
    ]
    adjf = np.asarray(adj, np.float32)
    in_maps = []
    for c in range(NCORES):
        b, j0 = c // 4, 128 * (c % 4)
        jselT = np.zeros((N, P), np.float32)
        jselT[j0 + np.arange(P), np.arange(P)] = 1.0
        jselT = pack_tiles(jselT, 4)  # (128, 512)
        adjTm = np.ascontiguousarray(adjf[b][:, j0 : j0 + P].T)  # (128, 512)
        in_maps.append(
            {
                "fT0": featT[b],
                "adjTm": adjTm,
                "jselT": jselT.astype(bf),
                "w0b": w0b,
                "w1b": w1b,
                "a1Ib": a1I,
                "a1Jpb": a1Jp,
                "a1br": a1br,
                "b0c": b0c,
                "b1c": b1c,
                "b0r": b0r,
                "b1r": b1r,
                "ident": ident,
                "onesb": np.ones((1, 64 * 512), np.float32).astype(bf),
            }
        )
    a2b = float(np.asarray(a2_b, np.float32).reshape(-1)[0])
    return in_maps, p_pos, a2b


def kernel(feature, adj, w0, b0, w1, b1, a1_w, a1_b, a2_w, a2_b, _trace=False):
    in_maps, p_pos, a2b = _prep_inputs(
        feature, adj, w0, b0, w1, b1, a1_w, a1_b, a2_w, a2_b
    )
    nc = _get_program(p_pos, a2b, debug=False)
    res = run_bass_kernel_spmd(
        nc, in_maps, core_ids=list(range(NCORES)), trace=_trace
    )
    out = np.stack(
        [
            np.asarray(res.results[0]["outT"], np.float32).T,
            np.asarray(res.results[4]["outT"], np.float32).T,
        ]
    )
    kernel._last_exec_time_ns = res.exec_time_ns
    kernel._last_profile = res.profile_json
    return out
